# revision 5
# baseline (speedup 1.0000x reference)
"""BiLSTM (T=2048, B=32, I=H=256) Bass kernel for 8 NeuronCores.

Sharding: data-parallel over batch. Core c handles batch lanes [4c:4c+4)
for BOTH directions (forward on x, backward on x flipped along time and
batch). All on-chip state is kept transposed ([H partitions, batch free])
so the sequential scan needs no per-step transposes:

  gates.T[4H, B] = Whh_perm.T.T @ h.T   (PE: stationary=Whh tile, moving=h.T)
  + xp.T (precomputed per chunk: Wih_perm @ x_t.T + biases)

Gate rows are permuted to [i, f, o, g] at build time so one Sigmoid op
covers i,f,o and one Tanh covers g. Length masking is exact and handled
on the host: a lane's post-length steps compute garbage that never
contaminates other lanes (batch lanes are independent columns end to
end), and the output tail t >= len is overwritten host-side with the
frozen value h[len-1] (identical to the reference's masked freeze).
"""

import sys

import numpy as np

# ---- problem constants (hardcoded per contract) ----
T, B, I, H = 2048, 32, 256, 256
NCORES = 8
NDIR = 2          # forward, backward
BL = B // NCORES  # 4 batch lanes per core per direction
B2 = 2 * BL       # (H-tile, lane) free width of h/c state
G = 8             # 4H / 128 gate row tiles, ordered [i0,i1,f0,f1,o0,o1,g0,g1]
KT = 2            # H / 128 contraction tiles
TC = 64           # scan chunk length (steps per For_i iteration)
NCHUNK = T // TC

_CACHE = {}


def _import_bass():
    try:
        import concourse.bass  # noqa: F401
    except ImportError:
        sys.path.insert(0, "/opt/trn_rl_repo")


def build_program(t_total=T, tc=TC, use_bf16_w=False):
    """Build the SPMD Bass program (identical on all cores)."""
    _import_bass()
    import concourse.bass as bass
    import concourse.mybir as mybir
    from concourse import bacc
    from concourse.tile import TileContext

    ds = bass.ds
    f32 = mybir.dt.float32
    dt_w = mybir.dt.bfloat16 if use_bf16_w else f32
    AF = mybir.ActivationFunctionType
    OP = mybir.AluOpType

    n_chunks = t_total // tc
    assert t_total % tc == 0 and tc * BL <= 512

    nc = bacc.Bacc("TRN2", target_bir_lowering=False, debug=False,
                   num_devices=NCORES)

    # DRAM I/O (2D layouts so dynamic row offsets are just kbase + const).
    # xarr rows per chunk: (dir, ki) -> 4 blocks of 128 rows.
    xarr = nc.dram_tensor("xarr", [n_chunks * NDIR * KT * 128, tc * BL], f32,
                          kind="ExternalInput")
    whhT = nc.dram_tensor("whhT", [NDIR * KT * G * 128, 128], dt_w,
                          kind="ExternalInput")
    wihT = nc.dram_tensor("wihT", [NDIR * KT * G * 128, 128], f32,
                          kind="ExternalInput")
    biasT = nc.dram_tensor("biasT", [128, NDIR * G], f32, kind="ExternalInput")
    hc0T = nc.dram_tensor("hc0T", [128, NDIR * 2 * B2], f32,
                          kind="ExternalInput")
    # hc_out rows per chunk: (dir, h/c) -> 4 blocks of 128 rows.
    hc_out = nc.dram_tensor("hc_out", [n_chunks * NDIR * 2 * 128, tc * B2],
                            f32, kind="ExternalOutput")

    from contextlib import ExitStack
    with TileContext(nc) as tc_ctx, ExitStack() as stk:
        tcx = tc_ctx
        wpool = stk.enter_context(tcx.tile_pool(name="weights", bufs=1))
        spool = stk.enter_context(tcx.tile_pool(name="state", bufs=1))
        xpool = stk.enter_context(tcx.tile_pool(name="xdata", bufs=1))
        tpool = stk.enter_context(tcx.tile_pool(name="temps", bufs=3))
        pgpool = stk.enter_context(tcx.tile_pool(name="psg", bufs=2,
                                                 space="PSUM"))
        papool = stk.enter_context(tcx.tile_pool(name="psa", bufs=4,
                                                 space="PSUM"))

        # --- persistent SBUF tensors ---
        whh_sb = wpool.tile([128, NDIR * KT * G * 128], dt_w)
        wih_sb = wpool.tile([128, NDIR * KT * G * 128], f32)
        bias_sb = wpool.tile([128, NDIR * G], f32)
        hc0_sb = wpool.tile([128, NDIR * 2 * B2], f32)
        h_hist = [spool.tile([128, (tc + 1) * B2], f32, tag=f"hh{d}",
                              name=f"h_hist{d}") for d in range(NDIR)]
        c_hist = [spool.tile([128, (tc + 1) * B2], f32, tag=f"ch{d}",
                              name=f"c_hist{d}") for d in range(NDIR)]
        xp = [xpool.tile([128, tc * G * BL], f32, tag=f"xp{d}",
                         name=f"xp{d}") for d in range(NDIR)]
        xin = xpool.tile([128, NDIR * KT * tc * BL], f32)

        def w_sl(sb, d, ki, j):
            off = ((d * KT + ki) * G + j) * 128
            return sb[:, off:off + 128]

        # --- load constants ---
        nasm = NDIR * KT * G
        nc.sync.dma_start(
            out=whh_sb[:].rearrange("p (a m) -> p a m", m=128),
            in_=whhT.ap().rearrange("(a p) m -> p a m", p=128))
        nc.sync.dma_start(
            out=wih_sb[:].rearrange("p (a m) -> p a m", m=128),
            in_=wihT.ap().rearrange("(a p) m -> p a m", p=128))
        nc.sync.dma_start(out=bias_sb[:], in_=biasT.ap())
        nc.sync.dma_start(out=hc0_sb[:], in_=hc0T.ap())
        for d in range(NDIR):
            nc.vector.tensor_copy(h_hist[d][:, 0:B2],
                                  hc0_sb[:, (d * 2 + 0) * B2:(d * 2 + 1) * B2])
            nc.vector.tensor_copy(c_hist[d][:, 0:B2],
                                  hc0_sb[:, (d * 2 + 1) * B2:(d * 2 + 2) * B2])

        def chunk_body(kbase):
            # kbase = chunk * (NDIR*KT*128) row offset into xarr;
            # also chunk * (NDIR*2*128) into hc_out (same stride: 512 rows).
            # 1) DMA x.T chunk in
            for d in range(NDIR):
                for ki in range(KT):
                    roff = (d * KT + ki) * 128
                    coff = (d * KT + ki) * tc * BL
                    nc.sync.dma_start(
                        out=xin[:, coff:coff + tc * BL],
                        in_=xarr.ap()[ds(kbase + roff, 128), :])
            # 2) Phase A: xp[d] = Wih_perm @ x.T + bias, per gate tile
            for d in range(NDIR):
                xpv = xp[d].rearrange("p (t g l) -> p t g l", g=G, l=BL)
                for j in range(G):
                    ps = papool.tile([128, tc * BL], f32, tag="pa")
                    for ki in range(KT):
                        coff = (d * KT + ki) * tc * BL
                        nc.tensor.matmul(ps[:],
                                         w_sl(wih_sb, d, ki, j),
                                         xin[:, coff:coff + tc * BL],
                                         start=(ki == 0), stop=(ki == KT - 1))
                    psv = ps.rearrange("p (t l) -> p t l", l=BL)
                    bcol = bias_sb[:, d * G + j:d * G + j + 1]
                    dst = xpv[:, :, j, :]
                    if j % 2 == 0:
                        nc.scalar.activation(dst, psv, AF.Identity, bias=bcol)
                    else:
                        nc.vector.tensor_scalar(dst, psv, bcol, None, OP.add)
            # 3) sequential scan
            for tl in range(tc):
                sl_prev = lambda dd, ki: h_hist[dd][:, tl * B2 + ki * BL:
                                                    tl * B2 + ki * BL + BL]
                ps_g = []
                for d in range(NDIR):
                    ps = pgpool.tile([128, G * BL], f32, tag=f"g{d}")
                    ps_g.append(ps)
                    for j in range(G):
                        for ki in range(KT):
                            nc.tensor.matmul(ps[:, j * BL:(j + 1) * BL],
                                             w_sl(whh_sb, d, ki, j),
                                             sl_prev(d, ki),
                                             start=(ki == 0),
                                             stop=(ki == KT - 1))
                for d in range(NDIR):
                    gsb = tpool.tile([128, G * BL], f32, tag=f"gs{d}")
                    xps = xp[d][:, tl * G * BL:(tl + 1) * G * BL]
                    nc.vector.scalar_tensor_tensor(gsb[:], ps_g[d][:], 0.0,
                                                   xps, OP.add, OP.add)
                    sig = tpool.tile([128, 6 * BL], f32, tag=f"sg{d}")
                    tg = tpool.tile([128, 2 * BL], f32, tag=f"tg{d}")
                    nc.scalar.activation(sig[:], gsb[:, 0:6 * BL], AF.Sigmoid)
                    nc.scalar.activation(tg[:], gsb[:, 6 * BL:8 * BL], AF.Tanh)
                    c_prev = c_hist[d][:, tl * B2:(tl + 1) * B2]
                    c_new = c_hist[d][:, (tl + 1) * B2:(tl + 2) * B2]
                    h_new = h_hist[d][:, (tl + 1) * B2:(tl + 2) * B2]
                    cf = tpool.tile([128, B2], f32, tag=f"cf{d}")
                    u = tpool.tile([128, B2], f32, tag=f"u{d}")
                    tcl = tpool.tile([128, B2], f32, tag=f"tc{d}")
                    nc.vector.tensor_mul(cf[:], sig[:, 2 * BL:4 * BL], c_prev)
                    nc.vector.tensor_mul(u[:], sig[:, 0:2 * BL], tg[:])
                    nc.vector.tensor_add(c_new, cf[:], u[:])
                    nc.scalar.activation(tcl[:], c_new, AF.Tanh)
                    nc.vector.tensor_mul(h_new, sig[:, 4 * BL:6 * BL], tcl[:])
            # 4) flush chunk outputs, carry state to slot 0
            for d in range(NDIR):
                nc.sync.dma_start(
                    out=hc_out.ap()[ds(kbase + (d * 2 + 0) * 128, 128), :],
                    in_=h_hist[d][:, B2:])
                nc.sync.dma_start(
                    out=hc_out.ap()[ds(kbase + (d * 2 + 1) * 128, 128), :],
                    in_=c_hist[d][:, B2:])
                nc.vector.tensor_copy(h_hist[d][:, 0:B2],
                                      h_hist[d][:, tc * B2:(tc + 1) * B2])
                nc.vector.tensor_copy(c_hist[d][:, 0:B2],
                                      c_hist[d][:, tc * B2:(tc + 1) * B2])

        if n_chunks == 1:
            chunk_body(0)
        else:
            import concourse.mybir as _mb
            with tcx.For_i(0, n_chunks * NDIR * KT * 128, NDIR * KT * 128,
                           hint_engines=(_mb.EngineType.PE,
                                         _mb.EngineType.Activation,
                                         _mb.EngineType.DVE)) as kbase:
                chunk_body(kbase)

    nc.compile()
    return nc


# ---------------- host-side data marshalling ----------------

def _perm_rows(w):
    """Reorder gate rows [i,f,g,o] -> [i,f,o,g]."""
    return np.concatenate([w[0:256], w[256:512], w[768:1024], w[512:768]], 0)


def prep_inputs(x, length, h0, c0, Wih_f, Whh_f, bih_f, bhh_f,
                Wih_b, Whh_b, bih_b, bhh_b, t_total=T, tc=TC,
                use_bf16_w=False):
    """Build per-core input dicts."""
    import ml_dtypes
    n_chunks = t_total // tc
    x = np.asarray(x, np.float32)
    x_b = x[::-1, ::-1, :]  # flip time and batch (torch.flip(input_, [0,1]))

    wihP = {0: _perm_rows(np.asarray(Wih_f)), 1: _perm_rows(np.asarray(Wih_b))}
    whhP = {0: _perm_rows(np.asarray(Whh_f)), 1: _perm_rows(np.asarray(Whh_b))}
    biasP = {0: _perm_rows((np.asarray(bih_f) + np.asarray(bhh_f))[:, None]),
             1: _perm_rows((np.asarray(bih_b) + np.asarray(bhh_b))[:, None])}

    def wtiles(w, dt):
        # [NDIR*KT*G*128, 128]: rows (d, ki, j, p) -> w[d].T tile
        out = np.empty((NDIR * KT * G * 128, 128), dt)
        for d in range(NDIR):
            wT = w[d].T.astype(dt)  # [H=256, 4H=1024]
            for ki in range(KT):
                for j in range(G):
                    off = ((d * KT + ki) * G + j) * 128
                    out[off:off + 128] = wT[ki * 128:(ki + 1) * 128,
                                            j * 128:(j + 1) * 128]
        return out

    whhT = wtiles(whhP, ml_dtypes.bfloat16 if use_bf16_w else np.float32)
    wihT = wtiles(wihP, np.float32)

    biasT = np.zeros((128, NDIR * G), np.float32)
    for d in range(NDIR):
        for j in range(G):
            biasT[:, d * G + j] = biasP[d][j * 128:(j + 1) * 128, 0]

    h0 = np.asarray(h0, np.float32)
    c0 = np.asarray(c0, np.float32)

    in_maps = []
    for core in range(NCORES):
        sl = slice(core * BL, (core + 1) * BL)
        # xarr
        xarr = np.empty((n_chunks * NDIR * KT * 128, tc * BL), np.float32)
        for d, xd in ((0, x), (1, x_b)):
            xs = xd[:t_total, sl, :]                    # [T, BL, I]
            xT = np.ascontiguousarray(xs.transpose(0, 2, 1))  # [T, I, BL]
            for k in range(n_chunks):
                for ki in range(KT):
                    roff = (k * NDIR * KT + d * KT + ki) * 128
                    blk = xT[k * tc:(k + 1) * tc, ki * 128:(ki + 1) * 128, :]
                    # [tc, 128, BL] -> [128, tc*BL]
                    xarr[roff:roff + 128] = (
                        blk.transpose(1, 0, 2).reshape(128, tc * BL))
        # hc0T: [128, (d, h/c, ki, lane)]
        hc0T = np.zeros((128, NDIR * 2 * B2), np.float32)
        for d in range(NDIR):
            for s, st in ((0, h0), (1, c0)):
                stT = st[sl].T  # [H, BL]
                for ki in range(KT):
                    off = (d * 2 + s) * B2 + ki * BL
                    hc0T[:, off:off + BL] = stT[ki * 128:(ki + 1) * 128, :]
        in_maps.append({"xarr": xarr, "whhT": whhT, "wihT": wihT,
                        "biasT": biasT, "hc0T": hc0T})
    return in_maps


def assemble_outputs(results, length, t_total=T, tc=TC):
    """results: list of per-core {'hc_out': ...}. Returns (output, cell)."""
    n_chunks = t_total // tc
    length = np.asarray(length)
    out_h = np.empty((t_total, 2 * B, H), np.float32)
    out_c = np.empty((t_total, 2 * B, H), np.float32)
    for core in range(NCORES):
        hc = results[core]["hc_out"].reshape(n_chunks, NDIR, 2, 128, tc, 2, BL)
        # [k, d, s, p, tl, ki, l] -> [d, s, k, tl, l, ki, p]
        hc = hc.transpose(1, 2, 0, 4, 6, 5, 3).reshape(
            NDIR, 2, t_total, BL, H)
        for d in range(NDIR):
            col0 = d * B + core * BL
            out_h[:, col0:col0 + BL, :] = hc[d, 0]
            out_c[:, col0:col0 + BL, :] = hc[d, 1]
    # host-side ragged tail freeze: for t >= len, output = value at len-1
    for b in range(B):
        ln = int(length[b])
        if ln < t_total:
            out_h[ln:, b] = out_h[ln - 1, b]
            out_c[ln:, b] = out_c[ln - 1, b]
            out_h[ln:, B + b] = out_h[ln - 1, B + b]
            out_c[ln:, B + b] = out_c[ln - 1, B + b]
    return out_h, out_c


def kernel(**inputs):
    _import_bass()
    from concourse.bass_utils import run_bass_kernel_spmd
    key = (T, TC, False)
    if key not in _CACHE:
        _CACHE[key] = build_program(T, TC, use_bf16_w=False)
    nc = _CACHE[key]
    in_maps = prep_inputs(**inputs)
    res = run_bass_kernel_spmd(nc, in_maps, list(range(NCORES)))
    out = assemble_outputs(res.results, inputs["length"])
    return out
